# revision 14
# baseline (speedup 1.0000x reference)
"""GCN layer (sparse COO matmul + 64x64 linear) on 8 TRN2 NeuronCores.

Strategy (per core, SPMD over 8 cores):
  - Nodes (output rows) are dest-sharded: core c owns dests [c*D, (c+1)*D).
  - Edges are bucketed host-side by (dest-window of 128, source-chunk of
    25000) and padded to 128-edge blocks; block counts are maxed across
    cores so one static program serves all 8 (SPMD).
  - X is stored bf16 feature-padded to 128 cols; source rows are fetched
    with SWDGE dma_gather (int16 chunk-local indices, 256B elems).
  - Per 128-edge block, a one-hot selector S[e, d] = (dest_e == d) * val_e
    is built on VectorE (is_equal vs an iota matrix, then scaled), and the
    segment-sum is one TensorE matmul accumulating into a PSUM tile per
    dest window.
  - The 64x64 linear runs per window: PE transpose of the aggregate, then
    agg @ W^T into PSUM, bias added on VectorE during the PSUM->SBUF copy.
"""
import os
import numpy as np
import ml_dtypes

import concourse.bacc as bacc
import concourse.mybir as mybir
from concourse.tile import TileContext
from concourse.bass_utils import run_bass_kernel_spmd

BF16 = ml_dtypes.bfloat16

N_NODES = 100000
N_EDGES = 1600000
D_FEAT = 64
NCORES = 8
CHUNK = 25000      # source rows per gather chunk (int16-addressable)
SW = 128           # dests per window (PSUM tile partition dim)
SPG = 7            # windows per superblock (gather-call granularity)


def _host_prep(L_rows, L_cols, L_vals, n_nodes, n_cores, chunk, sw):
    """Bucket/pad edges per core; build slot arrays + gather idx streams.

    Returns dict with the static structure (shared) and per-core arrays.
    """
    dper = n_nodes // n_cores
    nsw = (dper + sw - 1) // sw
    nchunk = (n_nodes + chunk - 1) // chunk

    rows = np.asarray(L_rows).astype(np.int64)
    cols = np.asarray(L_cols).astype(np.int64)
    vals = np.asarray(L_vals).astype(np.float32)

    core = rows // dper
    nbuck = nsw * nchunk

    per_core = []
    counts = np.zeros((n_cores, nbuck), dtype=np.int64)
    for c in range(n_cores):
        m = core == c
        rc, cc, vc = rows[m], cols[m], vals[m]
        dl = rc - c * dper
        swi = dl // sw
        dsub = (dl - swi * sw).astype(np.float32)
        k = cc // chunk
        il = (cc - k * chunk).astype(np.int64)
        bucket = swi * nchunk + k
        # secondary sort by source index: ascending HBM addresses within
        # each bucket give the gather DMA row-buffer locality
        order = np.lexsort((il, bucket))
        bucket = bucket[order]
        per_core.append((bucket, il[order], dsub[order], vc[order]))
        counts[c] = np.bincount(bucket, minlength=nbuck)

    nblk = (counts.max(axis=0) + 127) // 128          # [nbuck]
    nblk = nblk.reshape(nsw, nchunk)
    # every window needs >=1 block so its PSUM tile gets written
    empty_sw = nblk.sum(axis=1) == 0
    nblk[empty_sw, 0] = 1
    nblk_flat = nblk.reshape(-1)

    slot_start = np.zeros(nbuck + 1, dtype=np.int64)
    np.cumsum(128 * nblk_flat, out=slot_start[1:])
    tot_slots = int(slot_start[-1])
    tot_blk = tot_slots // 128

    core_arrays = []
    for c in range(n_cores):
        bucket, il, dsub, vc = per_core[c]
        n_c = np.bincount(bucket, minlength=nbuck)
        bstart = np.zeros(nbuck, dtype=np.int64)
        np.cumsum(n_c[:-1], out=bstart[1:])
        within = np.arange(len(bucket)) - bstart[bucket]
        slot = slot_start[bucket] + within

        il_s = np.zeros(tot_slots, dtype=np.int16)
        ds_s = np.zeros(tot_slots, dtype=np.float32)
        va_s = np.zeros(tot_slots, dtype=np.float32)
        il_s[slot] = il.astype(np.int16)
        ds_s[slot] = dsub
        va_s[slot] = vc

        dmeta = ds_s.reshape(tot_blk, 128).T.astype(BF16)   # [128, tot_blk]
        core_arrays.append((il_s, dmeta, va_s))

    return {
        "dper": dper, "nsw": nsw, "nchunk": nchunk, "chunk": chunk, "sw": sw,
        "nblk": nblk, "slot_start": slot_start,
        "tot_slots": tot_slots, "tot_blk": tot_blk,
        "core_arrays": core_arrays,
    }


def _build_calls(prep, spg):
    """Gather-call layout: one call per (superblock, chunk).

    Returns list of per-superblock dicts + total idx columns.
    """
    nsw, nchunk = prep["nsw"], prep["nchunk"]
    nblk, slot_start = prep["nblk"], prep["slot_start"]
    groups = []
    col0 = 0
    for g0 in range(0, nsw, spg):
        sws = list(range(g0, min(g0 + spg, nsw)))
        calls = []
        gcol0 = col0
        for k in range(nchunk):
            nbk = int(nblk[sws, k].sum())
            ni = 128 * nbk
            # slot ranges composing this call, in sw order
            ranges = [(int(slot_start[s * nchunk + k]),
                       int(slot_start[s * nchunk + k] + 128 * nblk[s, k]))
                      for s in sws]
            # call-relative block offset of each sw
            boff = {}
            acc = 0
            for s in sws:
                boff[s] = acc
                acc += int(nblk[s, k])
            calls.append({"k": k, "nbk": nbk, "ni": ni, "ranges": ranges,
                          "boff": boff, "col0": col0 - gcol0})
            col0 += ni // 16
        groups.append({"sws": sws, "calls": calls, "gcol0": gcol0,
                       "gcols": col0 - gcol0})
    return groups, col0


def _idx_stream(prep, groups, il_s):
    """Wrapped int16 index stream matching the gather-call layout."""
    out = np.zeros((128, groups[-1]["gcol0"] + groups[-1]["gcols"]),
                   dtype=np.int16)
    for g in groups:
        for call in g["calls"]:
            flat = np.concatenate([il_s[a:b] for a, b in call["ranges"]])
            w = flat.reshape(-1, 16).T                      # [16, ni/16]
            c0 = g["gcol0"] + call["col0"]
            out[:, c0:c0 + w.shape[1]] = np.tile(w, (8, 1))
    return out


def _build_program(prep, groups, totcols):
    nsw, nchunk = prep["nsw"], prep["nchunk"]
    nblk, slot_start = prep["nblk"], prep["slot_start"]
    dper, tot_blk = prep["dper"], prep["tot_blk"]
    chunk, sw = prep["chunk"], prep["sw"]
    max_nb = int(nblk.max())
    bf = mybir.dt.bfloat16
    f32 = mybir.dt.float32

    nc = bacc.Bacc("TRN2", num_swdge_queues=4)
    t_x = nc.dram_tensor("xbf", [chunk * nchunk, 128], bf, kind="ExternalInput")
    t_idx = nc.dram_tensor("idxs", [128, totcols], mybir.dt.int16,
                           kind="ExternalInput")
    t_dm = nc.dram_tensor("dmeta", [128, tot_blk], bf, kind="ExternalInput")
    t_vm = nc.dram_tensor("vmeta", [128, tot_blk], bf, kind="ExternalInput")
    t_io = nc.dram_tensor("iota2", [128, 128], bf, kind="ExternalInput")
    t_id = nc.dram_tensor("ident", [128, 128], bf, kind="ExternalInput")
    t_wt = nc.dram_tensor("wt", [64, 64], bf, kind="ExternalInput")
    t_bi = nc.dram_tensor("biasm", [128, 64], f32, kind="ExternalInput")
    t_out = nc.dram_tensor("out", [dper, 64], f32, kind="ExternalOutput")

    max_gcols = max(g["gcols"] for g in groups)
    max_nbk = [max(g["calls"][k]["nbk"] for g in groups) for k in range(nchunk)]

    with TileContext(nc) as tc:
        with (
            tc.tile_pool(name="const", bufs=1) as cpool,
            tc.tile_pool(name="idx", bufs=2) as ipool,
            tc.tile_pool(name="gath", bufs=2) as gpool,
            tc.tile_pool(name="sel", bufs=8) as spool,
            tc.tile_pool(name="agg", bufs=3) as apool,
            tc.tile_pool(name="outb", bufs=3) as opool,
            tc.tile_pool(name="ps", bufs=3, space="PSUM") as pspool,
            tc.tile_pool(name="pst", bufs=2, space="PSUM") as ptpool,
            tc.tile_pool(name="psf", bufs=2, space="PSUM") as pfpool,
        ):
            dm = cpool.tile([128, tot_blk], bf)
            vm = cpool.tile([128, tot_blk], bf)
            io2 = cpool.tile([128, 128], bf)
            idn = cpool.tile([128, 128], bf)
            wt = cpool.tile([64, 64], bf)
            bi = cpool.tile([128, 64], f32)
            nc.sync.dma_start(out=dm[:], in_=t_dm[:])
            nc.sync.dma_start(out=vm[:], in_=t_vm[:])
            nc.sync.dma_start(out=io2[:], in_=t_io[:])
            nc.sync.dma_start(out=idn[:], in_=t_id[:])
            nc.sync.dma_start(out=wt[:], in_=t_wt[:])
            nc.sync.dma_start(out=bi[:], in_=t_bi[:])

            for g in groups:
                idxt = ipool.tile([128, max_gcols], mybir.dt.int16, tag="idx")
                nc.sync.dma_start(
                    out=idxt[:, :g["gcols"]],
                    in_=t_idx[:, g["gcol0"]:g["gcol0"] + g["gcols"]])
                gts = []
                for k in range(nchunk):
                    call = g["calls"][k]
                    gt = gpool.tile([128, max(max_nbk[k], 1), 128], bf,
                                    tag=f"g{k}")
                    if call["ni"] > 0:
                        nc.gpsimd.dma_gather(
                            gt[:, :call["nbk"], :],
                            t_x[k * chunk:(k + 1) * chunk, :],
                            idxt[:, call["col0"]:call["col0"]
                                 + call["ni"] // 16],
                            call["ni"], call["ni"], 128,
                            single_packet=False, queue_num=k % 4)
                    gts.append(gt)

                # scale gathered rows by edge values, one op per call
                # (vmeta is laid out in gather-call order host-side)
                for k in range(nchunk):
                    call = g["calls"][k]
                    nbk = call["nbk"]
                    if nbk == 0:
                        continue
                    vb0 = (g["gcol0"] + call["col0"]) // 8
                    nc.vector.tensor_tensor(
                        out=gts[k][:, :nbk, 0:64],
                        in0=gts[k][:, :nbk, 0:64],
                        in1=vm[:, vb0:vb0 + nbk].to_broadcast(
                            [128, nbk, 64]),
                        op=mybir.AluOpType.mult)

                for s in g["sws"]:
                    # (k, j) matmul schedule for this window
                    sched = [(k, j) for k in range(nchunk)
                             for j in range(int(nblk[s, k]))]
                    psum = pspool.tile([128, 64], f32)
                    sels = {}
                    for k in range(nchunk):
                        nb = int(nblk[s, k])
                        if nb == 0:
                            continue
                        gblk0 = int(slot_start[s * nchunk + k]) // 128
                        sp = spool.tile([128, max_nb * 128], bf, tag="sel")
                        sp3 = sp[:, :nb * 128].rearrange(
                            "p (n d) -> p n d", d=128)
                        nc.vector.tensor_tensor(
                            out=sp3,
                            in0=io2[:].rearrange("p (a d) -> p a d", a=1)
                                .to_broadcast([128, nb, 128]),
                            in1=dm[:, gblk0:gblk0 + nb].to_broadcast(
                                [128, nb, 128]),
                            op=mybir.AluOpType.is_equal)
                        sels[k] = sp
                    for i, (k, j) in enumerate(sched):
                        call = g["calls"][k]
                        bb = call["boff"][s] + j
                        if os.environ.get("K_SKIP_MM"):
                            continue
                        nc.tensor.matmul(
                            psum[:],
                            lhsT=sels[k][:, j * 128:(j + 1) * 128],
                            rhs=gts[k][:, bb, 0:64],
                            start=(i == 0), stop=(i == len(sched) - 1))
                    if os.environ.get("K_SKIP_MM"):
                        nc.vector.memset(psum[:], 0.0)
                    r0 = s * sw
                    rows = min(sw, dper - r0)
                    if os.environ.get("K_SKIP_PHASE2"):
                        ob = opool.tile([128, 64], f32, tag="ob")
                        nc.vector.tensor_copy(out=ob[:], in_=psum[:])
                        nc.sync.dma_start(out=t_out[r0:r0 + rows, :],
                                          in_=ob[:rows, :])
                    else:
                        # linear layer: transpose agg, then agg @ W^T + b
                        aggb = apool.tile([128, 64], bf, tag="aggb")
                        nc.scalar.copy(out=aggb[:], in_=psum[:])
                        pst = ptpool.tile([64, 128], bf)
                        nc.tensor.transpose(pst[:], aggb[:], idn[:])
                        aggt = apool.tile([64, 128], bf, tag="aggt")
                        nc.scalar.copy(out=aggt[:], in_=pst[:])
                        psf = pfpool.tile([128, 64], f32)
                        nc.tensor.matmul(psf[:], lhsT=aggt[:], rhs=wt[:],
                                         start=True, stop=True)
                        ob = opool.tile([128, 64], f32, tag="ob")
                        nc.vector.tensor_tensor(out=ob[:], in0=psf[:],
                                                in1=bi[:],
                                                op=mybir.AluOpType.add)
                        nc.sync.dma_start(out=t_out[r0:r0 + rows, :],
                                          in_=ob[:rows, :])
    nc.compile()
    return nc


def _run(inputs, n_cores=NCORES, chunk=CHUNK, sw=SW, spg=SPG, trace=False):
    L_rows = inputs["L_rows"]
    L_cols = inputs["L_cols"]
    L_vals = inputs["L_vals"]
    X = np.asarray(inputs["X"], dtype=np.float32)
    W = np.asarray(inputs["W"], dtype=np.float32)
    b = np.asarray(inputs["b"], dtype=np.float32)
    n_nodes, d = X.shape

    prep = _host_prep(L_rows, L_cols, L_vals, n_nodes, n_cores, chunk, sw)
    groups, totcols = _build_calls(prep, spg)
    nc = _build_program(prep, groups, totcols)

    xbf = np.zeros((prep["nchunk"] * chunk, 128), dtype=BF16)
    xbf[:n_nodes, :d] = X.astype(BF16)
    iota2 = np.tile(np.arange(128, dtype=np.float32), (128, 1)).astype(BF16)
    ident = np.eye(128, dtype=np.float32).astype(BF16)
    wt = np.ascontiguousarray(W.T).astype(BF16)
    biasm = np.tile(b[None, :], (128, 1)).astype(np.float32)

    in_maps = []
    for c in range(n_cores):
        il_s, dmeta, va_s = prep["core_arrays"][c]
        # vmeta in gather-call order: one contiguous run of blocks per call
        parts = [va_s[a:b] for g in groups for call in g["calls"]
                 for a, b in call["ranges"]]
        vmeta2 = np.concatenate(parts).reshape(-1, 128).T.astype(BF16)
        in_maps.append({
            "xbf": xbf,
            "idxs": _idx_stream(prep, groups, il_s),
            "dmeta": np.ascontiguousarray(dmeta),
            "vmeta": np.ascontiguousarray(vmeta2),
            "iota2": iota2, "ident": ident, "wt": wt, "biasm": biasm,
        })
    res = run_bass_kernel_spmd(nc, in_maps, core_ids=list(range(n_cores)),
                               trace=False)
    out = np.concatenate([res.results[c]["out"] for c in range(n_cores)],
                         axis=0)
    return out, nc, in_maps


def kernel(L_rows, L_cols, L_vals, X, W, b):
    out, _, _ = _run({"L_rows": L_rows, "L_cols": L_cols, "L_vals": L_vals,
                      "X": X, "W": W, "b": b})
    return out



# revision 15
# speedup vs baseline: 1.1315x; 1.1315x over previous
"""GCN layer (sparse COO matmul + 64x64 linear) on 8 TRN2 NeuronCores.

Strategy (per core, SPMD over 8 cores):
  - Nodes (output rows) are dest-sharded: core c owns dests [c*D, (c+1)*D).
  - Edges are bucketed host-side by (dest-window of 128, source-chunk of
    25000) and padded to 128-edge blocks; block counts are maxed across
    cores so one static program serves all 8 (SPMD).
  - X is stored bf16 feature-padded to 128 cols; source rows are fetched
    with SWDGE dma_gather (int16 chunk-local indices, 256B elems).
  - Per 128-edge block, a one-hot selector S[e, d] = (dest_e == d) * val_e
    is built on VectorE (is_equal vs an iota matrix, then scaled), and the
    segment-sum is one TensorE matmul accumulating into a PSUM tile per
    dest window.
  - The 64x64 linear runs per window: PE transpose of the aggregate, then
    agg @ W^T into PSUM, bias added on VectorE during the PSUM->SBUF copy.
"""
import os
import numpy as np
import ml_dtypes

import concourse.bacc as bacc
import concourse.mybir as mybir
from concourse.tile import TileContext
from concourse.bass_utils import run_bass_kernel_spmd

BF16 = ml_dtypes.bfloat16

N_NODES = 100000
N_EDGES = 1600000
D_FEAT = 64
NCORES = 8
CHUNK = 25000      # source rows per gather chunk (int16-addressable)
SW = 128           # dests per window (PSUM tile partition dim)
SPG = 7            # windows per superblock (gather-call granularity)


def _host_prep(L_rows, L_cols, L_vals, n_nodes, n_cores, chunk, sw):
    """Bucket/pad edges per core; build slot arrays + gather idx streams.

    Returns dict with the static structure (shared) and per-core arrays.
    """
    dper = n_nodes // n_cores
    nsw = (dper + sw - 1) // sw
    nchunk = (n_nodes + chunk - 1) // chunk

    rows = np.asarray(L_rows).astype(np.int64)
    cols = np.asarray(L_cols).astype(np.int64)
    vals = np.asarray(L_vals).astype(np.float32)

    core = rows // dper
    nbuck = nsw * nchunk

    per_core = []
    counts = np.zeros((n_cores, nbuck), dtype=np.int64)
    for c in range(n_cores):
        m = core == c
        rc, cc, vc = rows[m], cols[m], vals[m]
        dl = rc - c * dper
        swi = dl // sw
        dsub = (dl - swi * sw).astype(np.float32)
        k = cc // chunk
        il = (cc - k * chunk).astype(np.int64)
        bucket = swi * nchunk + k
        # secondary sort by source index: ascending HBM addresses within
        # each bucket give the gather DMA row-buffer locality
        order = np.lexsort((il, bucket))
        bucket = bucket[order]
        per_core.append((bucket, il[order], dsub[order], vc[order]))
        counts[c] = np.bincount(bucket, minlength=nbuck)

    nblk = (counts.max(axis=0) + 127) // 128          # [nbuck]
    nblk = nblk.reshape(nsw, nchunk)
    # every window needs >=1 block so its PSUM tile gets written
    empty_sw = nblk.sum(axis=1) == 0
    nblk[empty_sw, 0] = 1
    nblk_flat = nblk.reshape(-1)

    slot_start = np.zeros(nbuck + 1, dtype=np.int64)
    np.cumsum(128 * nblk_flat, out=slot_start[1:])
    tot_slots = int(slot_start[-1])
    tot_blk = tot_slots // 128

    core_arrays = []
    for c in range(n_cores):
        bucket, il, dsub, vc = per_core[c]
        n_c = np.bincount(bucket, minlength=nbuck)
        bstart = np.zeros(nbuck, dtype=np.int64)
        np.cumsum(n_c[:-1], out=bstart[1:])
        within = np.arange(len(bucket)) - bstart[bucket]
        slot = slot_start[bucket] + within

        il_s = np.zeros(tot_slots, dtype=np.int16)
        ds_s = np.zeros(tot_slots, dtype=np.float32)
        va_s = np.zeros(tot_slots, dtype=np.float32)
        il_s[slot] = il.astype(np.int16)
        ds_s[slot] = dsub
        va_s[slot] = vc

        dmeta = ds_s.reshape(tot_blk, 128).T.astype(BF16)   # [128, tot_blk]
        core_arrays.append((il_s, dmeta, va_s))

    return {
        "dper": dper, "nsw": nsw, "nchunk": nchunk, "chunk": chunk, "sw": sw,
        "nblk": nblk, "slot_start": slot_start,
        "tot_slots": tot_slots, "tot_blk": tot_blk,
        "core_arrays": core_arrays,
    }


def _build_calls(prep, spg):
    """Gather-call layout: one call per (superblock, chunk).

    Returns list of per-superblock dicts + total idx columns.
    """
    nsw, nchunk = prep["nsw"], prep["nchunk"]
    nblk, slot_start = prep["nblk"], prep["slot_start"]
    groups = []
    col0 = 0
    for g0 in range(0, nsw, spg):
        sws = list(range(g0, min(g0 + spg, nsw)))
        calls = []
        gcol0 = col0
        for k in range(nchunk):
            nbk = int(nblk[sws, k].sum())
            ni = 128 * nbk
            # slot ranges composing this call, in sw order
            ranges = [(int(slot_start[s * nchunk + k]),
                       int(slot_start[s * nchunk + k] + 128 * nblk[s, k]))
                      for s in sws]
            # call-relative block offset of each sw
            boff = {}
            acc = 0
            for s in sws:
                boff[s] = acc
                acc += int(nblk[s, k])
            calls.append({"k": k, "nbk": nbk, "ni": ni, "ranges": ranges,
                          "boff": boff, "col0": col0 - gcol0})
            col0 += ni // 16
        groups.append({"sws": sws, "calls": calls, "gcol0": gcol0,
                       "gcols": col0 - gcol0})
    return groups, col0


def _idx_stream(prep, groups, il_s):
    """Wrapped int16 index stream matching the gather-call layout."""
    out = np.zeros((128, groups[-1]["gcol0"] + groups[-1]["gcols"]),
                   dtype=np.int16)
    for g in groups:
        for call in g["calls"]:
            flat = np.concatenate([il_s[a:b] for a, b in call["ranges"]])
            w = flat.reshape(-1, 16).T                      # [16, ni/16]
            c0 = g["gcol0"] + call["col0"]
            out[:, c0:c0 + w.shape[1]] = np.tile(w, (8, 1))
    return out


def _build_program(prep, groups, totcols):
    nsw, nchunk = prep["nsw"], prep["nchunk"]
    nblk, slot_start = prep["nblk"], prep["slot_start"]
    dper, tot_blk = prep["dper"], prep["tot_blk"]
    chunk, sw = prep["chunk"], prep["sw"]
    max_nb = int(nblk.max())
    bf = mybir.dt.bfloat16
    f32 = mybir.dt.float32

    nc = bacc.Bacc("TRN2", num_swdge_queues=4)
    t_x = nc.dram_tensor("xbf", [chunk * nchunk, 128], bf, kind="ExternalInput")
    t_idx = nc.dram_tensor("idxs", [128, totcols], mybir.dt.int16,
                           kind="ExternalInput")
    t_dm = nc.dram_tensor("dmeta", [128, tot_blk], bf, kind="ExternalInput")
    t_vm = nc.dram_tensor("vmeta", [128, tot_blk], bf, kind="ExternalInput")
    t_io = nc.dram_tensor("iota2", [128, 128], bf, kind="ExternalInput")
    t_id = nc.dram_tensor("ident", [128, 128], bf, kind="ExternalInput")
    t_wt = nc.dram_tensor("wt", [64, 64], bf, kind="ExternalInput")
    t_bi = nc.dram_tensor("biasm", [128, 64], f32, kind="ExternalInput")
    t_out = nc.dram_tensor("out", [dper, 64], f32, kind="ExternalOutput")

    max_gcols = max(g["gcols"] for g in groups)
    max_nbk = [max(g["calls"][k]["nbk"] for g in groups) for k in range(nchunk)]

    with TileContext(nc) as tc:
        with (
            tc.tile_pool(name="const", bufs=1) as cpool,
            tc.tile_pool(name="idx", bufs=3) as ipool,
            tc.tile_pool(name="gath", bufs=3) as gpool,
            tc.tile_pool(name="sel", bufs=8) as spool,
            tc.tile_pool(name="agg", bufs=3) as apool,
            tc.tile_pool(name="outb", bufs=3) as opool,
            tc.tile_pool(name="ps", bufs=3, space="PSUM") as pspool,
            tc.tile_pool(name="pst", bufs=2, space="PSUM") as ptpool,
            tc.tile_pool(name="psf", bufs=2, space="PSUM") as pfpool,
        ):
            dm = cpool.tile([128, tot_blk], bf)
            vm = cpool.tile([128, tot_blk], bf)
            io2 = cpool.tile([128, 128], bf)
            idn = cpool.tile([128, 128], bf)
            wt = cpool.tile([64, 64], bf)
            bi = cpool.tile([128, 64], f32)
            nc.sync.dma_start(out=dm[:], in_=t_dm[:])
            nc.sync.dma_start(out=vm[:], in_=t_vm[:])
            nc.sync.dma_start(out=io2[:], in_=t_io[:])
            nc.sync.dma_start(out=idn[:], in_=t_id[:])
            nc.sync.dma_start(out=wt[:], in_=t_wt[:])
            nc.sync.dma_start(out=bi[:], in_=t_bi[:])

            for g in groups:
                idxt = ipool.tile([128, max_gcols], mybir.dt.int16, tag="idx")
                nc.sync.dma_start(
                    out=idxt[:, :g["gcols"]],
                    in_=t_idx[:, g["gcol0"]:g["gcol0"] + g["gcols"]])
                gts = []
                for k in range(nchunk):
                    call = g["calls"][k]
                    gt = gpool.tile([128, max(max_nbk[k], 1), 128], bf,
                                    tag=f"g{k}")
                    if call["ni"] > 0:
                        nc.gpsimd.dma_gather(
                            gt[:, :call["nbk"], :],
                            t_x[k * chunk:(k + 1) * chunk, :],
                            idxt[:, call["col0"]:call["col0"]
                                 + call["ni"] // 16],
                            call["ni"], call["ni"], 128,
                            single_packet=False, queue_num=k % 4)
                    gts.append(gt)

                # scale gathered rows by edge values, one op per call
                # (vmeta is laid out in gather-call order host-side)
                for k in range(nchunk):
                    call = g["calls"][k]
                    nbk = call["nbk"]
                    if nbk == 0:
                        continue
                    vb0 = (g["gcol0"] + call["col0"]) // 8
                    nc.vector.tensor_tensor(
                        out=gts[k][:, :nbk, 0:64],
                        in0=gts[k][:, :nbk, 0:64],
                        in1=vm[:, vb0:vb0 + nbk].to_broadcast(
                            [128, nbk, 64]),
                        op=mybir.AluOpType.mult)

                for s in g["sws"]:
                    # (k, j) matmul schedule for this window
                    sched = [(k, j) for k in range(nchunk)
                             for j in range(int(nblk[s, k]))]
                    psum = pspool.tile([128, 64], f32)
                    sels = {}
                    for k in range(nchunk):
                        nb = int(nblk[s, k])
                        if nb == 0:
                            continue
                        gblk0 = int(slot_start[s * nchunk + k]) // 128
                        sp = spool.tile([128, max_nb * 128], bf, tag="sel")
                        sp3 = sp[:, :nb * 128].rearrange(
                            "p (n d) -> p n d", d=128)
                        nc.vector.tensor_tensor(
                            out=sp3,
                            in0=io2[:].rearrange("p (a d) -> p a d", a=1)
                                .to_broadcast([128, nb, 128]),
                            in1=dm[:, gblk0:gblk0 + nb].to_broadcast(
                                [128, nb, 128]),
                            op=mybir.AluOpType.is_equal)
                        sels[k] = sp
                    for i, (k, j) in enumerate(sched):
                        call = g["calls"][k]
                        bb = call["boff"][s] + j
                        if os.environ.get("K_SKIP_MM"):
                            continue
                        nc.tensor.matmul(
                            psum[:],
                            lhsT=sels[k][:, j * 128:(j + 1) * 128],
                            rhs=gts[k][:, bb, 0:64],
                            start=(i == 0), stop=(i == len(sched) - 1))
                    if os.environ.get("K_SKIP_MM"):
                        nc.vector.memset(psum[:], 0.0)
                    r0 = s * sw
                    rows = min(sw, dper - r0)
                    if os.environ.get("K_SKIP_PHASE2"):
                        ob = opool.tile([128, 64], f32, tag="ob")
                        nc.vector.tensor_copy(out=ob[:], in_=psum[:])
                        nc.sync.dma_start(out=t_out[r0:r0 + rows, :],
                                          in_=ob[:rows, :])
                    else:
                        # linear layer: transpose agg, then agg @ W^T + b
                        aggb = apool.tile([128, 64], bf, tag="aggb")
                        nc.scalar.copy(out=aggb[:], in_=psum[:])
                        pst = ptpool.tile([64, 128], bf)
                        nc.tensor.transpose(pst[:], aggb[:], idn[:])
                        aggt = apool.tile([64, 128], bf, tag="aggt")
                        nc.scalar.copy(out=aggt[:], in_=pst[:])
                        psf = pfpool.tile([128, 64], f32)
                        nc.tensor.matmul(psf[:], lhsT=aggt[:], rhs=wt[:],
                                         start=True, stop=True)
                        ob = opool.tile([128, 64], f32, tag="ob")
                        nc.vector.tensor_tensor(out=ob[:], in0=psf[:],
                                                in1=bi[:],
                                                op=mybir.AluOpType.add)
                        nc.sync.dma_start(out=t_out[r0:r0 + rows, :],
                                          in_=ob[:rows, :])
    nc.compile()
    return nc


def _run(inputs, n_cores=NCORES, chunk=CHUNK, sw=SW, spg=SPG, trace=False):
    L_rows = inputs["L_rows"]
    L_cols = inputs["L_cols"]
    L_vals = inputs["L_vals"]
    X = np.asarray(inputs["X"], dtype=np.float32)
    W = np.asarray(inputs["W"], dtype=np.float32)
    b = np.asarray(inputs["b"], dtype=np.float32)
    n_nodes, d = X.shape

    prep = _host_prep(L_rows, L_cols, L_vals, n_nodes, n_cores, chunk, sw)
    groups, totcols = _build_calls(prep, spg)
    nc = _build_program(prep, groups, totcols)

    xbf = np.zeros((prep["nchunk"] * chunk, 128), dtype=BF16)
    xbf[:n_nodes, :d] = X.astype(BF16)
    iota2 = np.tile(np.arange(128, dtype=np.float32), (128, 1)).astype(BF16)
    ident = np.eye(128, dtype=np.float32).astype(BF16)
    wt = np.ascontiguousarray(W.T).astype(BF16)
    biasm = np.tile(b[None, :], (128, 1)).astype(np.float32)

    in_maps = []
    for c in range(n_cores):
        il_s, dmeta, va_s = prep["core_arrays"][c]
        # vmeta in gather-call order: one contiguous run of blocks per call
        parts = [va_s[a:b] for g in groups for call in g["calls"]
                 for a, b in call["ranges"]]
        vmeta2 = np.concatenate(parts).reshape(-1, 128).T.astype(BF16)
        in_maps.append({
            "xbf": xbf,
            "idxs": _idx_stream(prep, groups, il_s),
            "dmeta": np.ascontiguousarray(dmeta),
            "vmeta": np.ascontiguousarray(vmeta2),
            "iota2": iota2, "ident": ident, "wt": wt, "biasm": biasm,
        })
    res = run_bass_kernel_spmd(nc, in_maps, core_ids=list(range(n_cores)),
                               trace=False)
    out = np.concatenate([res.results[c]["out"] for c in range(n_cores)],
                         axis=0)
    return out, nc, in_maps


def kernel(L_rows, L_cols, L_vals, X, W, b):
    out, _, _ = _run({"L_rows": L_rows, "L_cols": L_cols, "L_vals": L_vals,
                      "X": X, "W": W, "b": b})
    return out



# revision 30
# speedup vs baseline: 2.2532x; 1.9913x over previous
"""GCN layer (sparse COO matmul + 64x64 linear) on 8 TRN2 NeuronCores.

Strategy (per core, SPMD over 8 cores):
  - Nodes (output rows) are dest-sharded: core c owns dests [c*D, (c+1)*D).
  - Edges are grouped host-side by gather call = (superblock of SPG dest
    windows, source-chunk of 25000) and packed DENSELY within each call
    (padding only at the call tail, to a 128 multiple, maxed across cores
    so one static program serves all 8). 128-slot blocks may straddle
    window boundaries; each (window, call) selector masks foreign slots
    with a sentinel dest (200) so is_equal yields zero columns for them.
  - The matmul block range of each (window, chunk) is the UNION over the 8
    cores of the blocks its edges touch (boundaries differ per core; the
    selector mask makes the extra matmuls no-ops).
  - X is stored bf16 feature-padded to 128 cols; source rows are fetched
    with SWDGE dma_gather (int16 chunk-local indices, 256B elems).
  - Per block, a one-hot selector S[e, d] = (dest_e == d) is built on
    VectorE (is_equal vs an iota matrix); the edge values are multiplied
    into the gathered rows (one op per gather call), and the segment-sum
    is one TensorE matmul per block accumulating into the window's PSUM.
  - dmeta/vmeta are bf16 and host-expanded x4 so every DVE operand has a
    packed (stride-1) innermost AP dim - this enables the fast 16-bit DVE
    mode (~1.7x on the selector builds, measured).
  - The gather datapath is SWDGE-ring latency-bound: ~32.5 GB/s per queue,
    ~86-98 GB/s with all 4 queues (measured). Calls are spread round-robin
    over the 4 queues; SPG=7 empirically minimizes ring-wait stalls.
  - The 64x64 linear runs per window: PE transpose of the aggregate, then
    agg @ W^T into PSUM, bias added on VectorE during the PSUM->SBUF copy.
"""
import os
import numpy as np
import ml_dtypes

import concourse.bacc as bacc
import concourse.mybir as mybir
from concourse.tile import TileContext
from concourse.bass_utils import run_bass_kernel_spmd

BF16 = ml_dtypes.bfloat16

N_NODES = 100000
N_EDGES = 1600000
D_FEAT = 64
NCORES = 8
CHUNK = 25000      # source rows per gather chunk (int16-addressable)
SW = 128           # dests per window (PSUM tile partition dim)
SPG = 7            # windows per superblock (gather-call granularity)
SENT = np.float32(200.0)   # sentinel dest: never matches iota 0..127


def _host_prep(L_rows, L_cols, L_vals, n_nodes, n_cores, chunk, sw, spg):
    """Dense per-call packing + shared (union) matmul schedule."""
    dper = n_nodes // n_cores
    nsw = (dper + sw - 1) // sw
    nchunk = (n_nodes + chunk - 1) // chunk
    ngroups = (nsw + spg - 1) // spg
    ncalls = ngroups * nchunk
    nbuck = nsw * nchunk

    rows = np.asarray(L_rows).astype(np.int64)
    cols = np.asarray(L_cols).astype(np.int64)
    vals = np.asarray(L_vals).astype(np.float32)
    core = rows // dper

    per_core = []
    counts = np.zeros((n_cores, nbuck), dtype=np.int64)
    call_tot = np.zeros((n_cores, ncalls), dtype=np.int64)
    for c in range(n_cores):
        m = core == c
        rc, cc, vc = rows[m], cols[m], vals[m]
        dl = rc - c * dper
        swi = dl // sw
        dsub = (dl - swi * sw).astype(np.float32)
        k = cc // chunk
        il = (cc - k * chunk).astype(np.int64)
        callid = (swi // spg) * nchunk + k
        order = np.lexsort((dsub, swi, callid))
        per_core.append((callid[order], swi[order], il[order],
                         dsub[order], vc[order]))
        counts[c] = np.bincount(swi * nchunk + k, minlength=nbuck)
        call_tot[c] = np.bincount(callid, minlength=ncalls)

    # shared per-call block counts (padding only at the call tail)
    nbk = np.maximum((call_tot.max(axis=0) + 127) // 128, 1)   # [ncalls]
    call_blk0 = np.zeros(ncalls + 1, dtype=np.int64)
    np.cumsum(nbk, out=call_blk0[1:])
    tot_blk = int(call_blk0[-1])
    tot_slots = 128 * tot_blk

    # union (over cores) block range of each (window, chunk) within its call
    los = np.full((nsw, nchunk), 1 << 30, dtype=np.int64)
    his = np.full((nsw, nchunk), -1, dtype=np.int64)
    for c in range(n_cores):
        for g in range(ngroups):
            sws = range(g * spg, min((g + 1) * spg, nsw))
            for k in range(nchunk):
                off = 0
                for s in sws:
                    cnt = int(counts[c][s * nchunk + k])
                    if cnt > 0:
                        los[s, k] = min(los[s, k], off // 128)
                        his[s, k] = max(his[s, k], (off + cnt + 127) // 128)
                    off += cnt
    for s in range(nsw):
        if (his[s] < 0).all():       # window empty on all cores
            los[s, 0], his[s, 0] = 0, 1

    # selector column layout: per (s, k) with a non-empty range
    selcols = {}
    C = 0
    for g in range(ngroups):
        sws = range(g * spg, min((g + 1) * spg, nsw))
        for k in range(nchunk):
            for s in sws:
                if his[s, k] > 0 and his[s, k] > los[s, k]:
                    nb = int(his[s, k] - los[s, k])
                    selcols[(s, k)] = (C, int(los[s, k]), nb)
                    C += nb

    core_arrays = []
    for c in range(n_cores):
        cid, swi_s, il_e, ds_e, va_e = per_core[c]
        coff = np.zeros(ncalls, dtype=np.int64)
        coff[1:] = np.cumsum(call_tot[c])[:-1]
        within = np.arange(len(cid)) - coff[cid]
        slot = 128 * call_blk0[cid] + within

        il_s = np.zeros(tot_slots, dtype=np.int16)
        ds_s = np.zeros(tot_slots, dtype=np.float32)
        win_s = np.full(tot_slots, -1, dtype=np.int64)
        va_s = np.zeros(tot_slots, dtype=np.float32)
        il_s[slot] = il_e.astype(np.int16)
        ds_s[slot] = ds_e
        win_s[slot] = swi_s
        va_s[slot] = va_e

        dmeta = np.full((128, C), SENT, dtype=np.float32)
        for (s, k), (c0, lo, nb) in selcols.items():
            call = (s // spg) * nchunk + k
            base = 128 * (call_blk0[call] + lo)
            bv = ds_s[base:base + 128 * nb].reshape(nb, 128).T
            bw = win_s[base:base + 128 * nb].reshape(nb, 128).T
            dmeta[:, c0:c0 + nb] = np.where(bw == s, bv, SENT)
        dmeta4 = np.repeat(dmeta.astype(BF16), 4, axis=1)
        vmeta4 = np.repeat(
            va_s.reshape(tot_blk, 128).T.astype(BF16), 4, axis=1)
        core_arrays.append((il_s, dmeta4, vmeta4))

    # group/call structure for the program
    groups = []
    col0 = 0
    for g in range(ngroups):
        sws = list(range(g * spg, min((g + 1) * spg, nsw)))
        calls = []
        gcol0 = col0
        for k in range(nchunk):
            cid2 = g * nchunk + k
            nb_c = int(nbk[cid2])
            calls.append({"k": k, "nbk": nb_c, "ni": 128 * nb_c,
                          "blk0": int(call_blk0[cid2]),
                          "col0": col0 - gcol0})
            col0 += 128 * nb_c // 16
        groups.append({"sws": sws, "calls": calls, "gcol0": gcol0,
                       "gcols": col0 - gcol0})

    return {
        "dper": dper, "nsw": nsw, "nchunk": nchunk, "chunk": chunk, "sw": sw,
        "tot_slots": tot_slots, "tot_blk": tot_blk, "C": C,
        "selcols": selcols, "groups": groups, "totcols": col0,
        "max_selnb": max(nb for _, _, nb in selcols.values()),
        "core_arrays": core_arrays,
    }


def _idx_stream(prep, il_s):
    """Wrapped int16 index stream matching the gather-call layout."""
    groups = prep["groups"]
    out = np.zeros((128, prep["totcols"]), dtype=np.int16)
    for g in groups:
        for call in g["calls"]:
            a = 128 * call["blk0"]
            w = il_s[a:a + call["ni"]].reshape(-1, 16).T    # [16, ni/16]
            c0 = g["gcol0"] + call["col0"]
            out[:, c0:c0 + w.shape[1]] = np.tile(w, (8, 1))
    return out


def _build_program(prep):
    nsw, nchunk = prep["nsw"], prep["nchunk"]
    dper, tot_blk, C = prep["dper"], prep["tot_blk"], prep["C"]
    chunk, sw = prep["chunk"], prep["sw"]
    groups, selcols = prep["groups"], prep["selcols"]
    max_nb = prep["max_selnb"]
    bf = mybir.dt.bfloat16
    f32 = mybir.dt.float32

    nc = bacc.Bacc("TRN2", num_swdge_queues=4)
    t_x = nc.dram_tensor("xbf", [chunk * nchunk, 128], bf, kind="ExternalInput")
    t_idx = nc.dram_tensor("idxs", [128, prep["totcols"]], mybir.dt.int16,
                           kind="ExternalInput")
    t_dm = nc.dram_tensor("dmeta", [128, 4 * C], bf, kind="ExternalInput")
    t_vm = nc.dram_tensor("vmeta", [128, 4 * tot_blk], bf,
                          kind="ExternalInput")
    t_io = nc.dram_tensor("iota2", [128, 128], bf, kind="ExternalInput")
    t_id = nc.dram_tensor("ident", [128, 128], bf, kind="ExternalInput")
    t_wt = nc.dram_tensor("wt", [64, 64], bf, kind="ExternalInput")
    t_bi = nc.dram_tensor("biasm", [128, 64], f32, kind="ExternalInput")
    t_out = nc.dram_tensor("out", [dper, 64], f32, kind="ExternalOutput")

    max_gcols = max(g["gcols"] for g in groups)
    max_nbk = [max(g["calls"][k]["nbk"] for g in groups)
               for k in range(nchunk)]

    with TileContext(nc) as tc:
        with (
            tc.tile_pool(name="const", bufs=1) as cpool,
            tc.tile_pool(name="idx", bufs=3) as ipool,
            tc.tile_pool(name="gath", bufs=3) as gpool,
            tc.tile_pool(name="sel", bufs=8) as spool,
            tc.tile_pool(name="agg", bufs=3) as apool,
            tc.tile_pool(name="outb", bufs=3) as opool,
            tc.tile_pool(name="ps", bufs=3, space="PSUM") as pspool,
            tc.tile_pool(name="pst", bufs=2, space="PSUM") as ptpool,
            tc.tile_pool(name="psf", bufs=2, space="PSUM") as pfpool,
        ):
            dm = cpool.tile([128, 4 * C], bf)
            vm = cpool.tile([128, 4 * tot_blk], bf)
            io2 = cpool.tile([128, 128], bf)
            idn = cpool.tile([128, 128], bf)
            wt = cpool.tile([64, 64], bf)
            bi = cpool.tile([128, 64], f32)
            nc.sync.dma_start(out=dm[:], in_=t_dm[:])
            nc.sync.dma_start(out=vm[:], in_=t_vm[:])
            nc.sync.dma_start(out=io2[:], in_=t_io[:])
            nc.sync.dma_start(out=idn[:], in_=t_id[:])
            nc.sync.dma_start(out=wt[:], in_=t_wt[:])
            nc.sync.dma_start(out=bi[:], in_=t_bi[:])

            for g in groups:
                idxt = ipool.tile([128, max_gcols], mybir.dt.int16, tag="idx")
                nc.sync.dma_start(
                    out=idxt[:, :g["gcols"]],
                    in_=t_idx[:, g["gcol0"]:g["gcol0"] + g["gcols"]])
                gts = []
                for k in range(nchunk):
                    call = g["calls"][k]
                    gt = gpool.tile([128, max_nbk[k], 128], bf, tag=f"g{k}")
                    nc.gpsimd.dma_gather(
                        gt[:, :call["nbk"], :],
                        t_x[k * chunk:(k + 1) * chunk, :],
                        idxt[:, call["col0"]:call["col0"]
                             + call["ni"] // 16],
                        call["ni"], call["ni"], 128,
                        single_packet=False, queue_num=k % 4)
                    gts.append(gt)

                # scale gathered rows by edge values, one op per call
                # (vmeta is in gather-call order = slot order)
                for k in range(nchunk):
                    call = g["calls"][k]
                    nbk_c = call["nbk"]
                    vb0 = call["blk0"]
                    gt4 = gts[k][:, :nbk_c, 0:64].rearrange(
                        "p n (e q) -> p n e q", q=4)
                    vm4 = vm[:, 4 * vb0:4 * (vb0 + nbk_c)].rearrange(
                        "p (n a q) -> p n a q", a=1, q=4).to_broadcast(
                        [128, nbk_c, 16, 4])
                    nc.vector.tensor_tensor(
                        out=gt4, in0=gt4, in1=vm4,
                        op=mybir.AluOpType.mult)

                for s in g["sws"]:
                    sched = [(k, j) for k in range(nchunk)
                             if (s, k) in selcols
                             for j in range(selcols[(s, k)][1],
                                            selcols[(s, k)][1]
                                            + selcols[(s, k)][2])]
                    psum = pspool.tile([128, 64], f32)
                    sels = {}
                    for k in range(nchunk):
                        if (s, k) not in selcols:
                            continue
                        c0, lo, nb = selcols[(s, k)]
                        sp = spool.tile([128, max_nb * 128], bf, tag="sel")
                        sp4 = sp[:, :nb * 128].rearrange(
                            "p (n e q) -> p n e q", n=nb, e=32, q=4)
                        io4 = io2[:].rearrange(
                            "p (a e q) -> p a e q", a=1, e=32, q=4) \
                            .to_broadcast([128, nb, 32, 4])
                        dm4 = dm[:, 4 * c0:4 * (c0 + nb)].rearrange(
                            "p (n a q) -> p n a q", a=1, q=4).to_broadcast(
                            [128, nb, 32, 4])
                        nc.vector.tensor_tensor(
                            out=sp4, in0=io4, in1=dm4,
                            op=mybir.AluOpType.is_equal)
                        sels[k] = sp
                    for i, (k, j) in enumerate(sched):
                        if os.environ.get("K_SKIP_MM"):
                            continue
                        lo = selcols[(s, k)][1]
                        nc.tensor.matmul(
                            psum[:],
                            lhsT=sels[k][:, (j - lo) * 128:
                                         (j - lo + 1) * 128],
                            rhs=gts[k][:, j, 0:64],
                            start=(i == 0), stop=(i == len(sched) - 1))
                    if os.environ.get("K_SKIP_MM"):
                        nc.vector.memset(psum[:], 0.0)
                    r0 = s * sw
                    rows = min(sw, dper - r0)
                    if os.environ.get("K_SKIP_PHASE2"):
                        ob = opool.tile([128, 64], f32, tag="ob")
                        nc.vector.tensor_copy(out=ob[:], in_=psum[:])
                        nc.sync.dma_start(out=t_out[r0:r0 + rows, :],
                                          in_=ob[:rows, :])
                    else:
                        # linear layer: transpose agg, then agg @ W^T + b
                        aggb = apool.tile([128, 64], bf, tag="aggb")
                        nc.scalar.copy(out=aggb[:], in_=psum[:])
                        pst = ptpool.tile([64, 128], bf)
                        nc.tensor.transpose(pst[:], aggb[:], idn[:])
                        aggt = apool.tile([64, 128], bf, tag="aggt")
                        nc.scalar.copy(out=aggt[:], in_=pst[:])
                        psf = pfpool.tile([128, 64], f32)
                        nc.tensor.matmul(psf[:], lhsT=aggt[:], rhs=wt[:],
                                         start=True, stop=True)
                        ob = opool.tile([128, 64], f32, tag="ob")
                        nc.vector.tensor_tensor(out=ob[:], in0=psf[:],
                                                in1=bi[:],
                                                op=mybir.AluOpType.add)
                        nc.sync.dma_start(out=t_out[r0:r0 + rows, :],
                                          in_=ob[:rows, :])
    nc.compile()
    return nc


def _run(inputs, n_cores=NCORES, chunk=CHUNK, sw=SW, spg=SPG, trace=False):
    L_rows = inputs["L_rows"]
    L_cols = inputs["L_cols"]
    L_vals = inputs["L_vals"]
    X = np.asarray(inputs["X"], dtype=np.float32)
    W = np.asarray(inputs["W"], dtype=np.float32)
    b = np.asarray(inputs["b"], dtype=np.float32)
    n_nodes, d = X.shape

    prep = _host_prep(L_rows, L_cols, L_vals, n_nodes, n_cores, chunk, sw,
                      spg)
    nc = _build_program(prep)

    xbf = np.zeros((prep["nchunk"] * chunk, 128), dtype=BF16)
    xbf[:n_nodes, :d] = X.astype(BF16)
    iota2 = np.tile(np.arange(128, dtype=np.float32), (128, 1)).astype(BF16)
    ident = np.eye(128, dtype=np.float32).astype(BF16)
    wt = np.ascontiguousarray(W.T).astype(BF16)
    biasm = np.tile(b[None, :], (128, 1)).astype(np.float32)

    in_maps = []
    for c in range(n_cores):
        il_s, dmeta4, vmeta4 = prep["core_arrays"][c]
        in_maps.append({
            "xbf": xbf,
            "idxs": _idx_stream(prep, il_s),
            "dmeta": np.ascontiguousarray(dmeta4),
            "vmeta": np.ascontiguousarray(vmeta4),
            "iota2": iota2, "ident": ident, "wt": wt, "biasm": biasm,
        })
    res = run_bass_kernel_spmd(nc, in_maps, core_ids=list(range(n_cores)),
                               trace=False)
    out = np.concatenate([res.results[c]["out"] for c in range(n_cores)],
                         axis=0)
    return out, nc, in_maps


def kernel(L_rows, L_cols, L_vals, X, W, b):
    out, _, _ = _run({"L_rows": L_rows, "L_cols": L_cols, "L_vals": L_vals,
                      "X": X, "W": W, "b": b})
    return out
